# revision 44
# baseline (speedup 1.0000x reference)
"""Trainium2 Bass kernel for BCE + distance-decorrelation (DisCo) loss.

Reference math (N = 8192):
    bce  = mean((softplus(o) - o*l) * w)
    nw   = w * N / sum(w)
    a_ij = |o_i - o_j|, b_ij = |e_i - e_j|
    u_i  = (1/N) sum_j a_ij nw_j
    A    = a - u_j - u_i + mA,  mA = (1/N) sum nw u    (same for B with v, mB)
    num  = (1/N^2) sum_ij nw_i nw_j A_ij B_ij
    den  = [(1/N^2) sum nw nw A^2] [(1/N^2) sum nw nw B^2]
    disco = num / sqrt(den);  tot = bce + 0.1 * disco

Sorted-sign scheme: all quantities entering the final scalars are invariant
under a common permutation of the N samples, so the host sorts by o. In
sorted order |o_i - o_j| = sigma_ij (o_i - o_j) with sigma_ij = sign(i - j)
POSITIONAL (ties are safe: both sides are 0). Hence the only O(N^2) row sums

    st_i = sum_j om_j |o_i-o_j||e_i-e_j| = o_i G_i - H_i
    G_i  = sum_j sigma_ij om_j b_ij
    H_i  = sum_j sigma_ij (om_j o_j) b_ij
    sv_i = sum_j om_j b_ij

are all weighted row sums of the SINGLE matrix b = |e_i - e_j|, with signs
constant per 128-row j-block (baked into the matmul stationary on the host)
except at the 8 diagonal blocks per core, fixed up with a constant strict
lower-triangular mask:  strip = -full + 2 * (mask * b)  (diag of b is 0).
The a-side row sums su_i reduce to exact f64 prefix sums on the host, and
T_aa/T_bb have exact closed forms; the generic centering identity
    num*N^2  = T_ab + (2S-4N) P_uv + (4N^2-4NS+S^2) mA mB
(valid for ANY symmetric matrices consistent with their row sums) finishes
the computation on the host in f64 from O(N) device partials.

Device work per core per j-block (64 blocks of 128 j's x 1024 i's), fp16:
    b[j, i] = |e_i - e_j| via ACT Abs-with-bias (1 instr, ~1.37us), or
    DVE tensor_scalar add + sign-clear AND (2 instr, ~1.08us), with the AND
    offloaded to the Pool engine on some blocks (engine balance).
    one 4-col fp16 matmul per 512-chunk: stationary [s*om, s*mhi, s*mlo, om]
    (m = om*o split hi+lo; lo scaled 2^10 to dodge fp16 underflow)
accumulating into 2 psum col-groups (one per chunk). Per-core j-block
PERMUTATION puts the core's 8 diagonal blocks at positions p%8==0, keeping
the SPMD program core-independent.
"""

from contextlib import ExitStack

import numpy as np

import concourse.bacc as bacc
import concourse.bass as bass
import concourse.tile as tile
from concourse import mybir
from concourse.bass_utils import run_bass_kernel_spmd

N = 8192
NCORES = 8
P = 128
LAM = 0.1

F32 = mybir.dt.float32
GEN_DT = mybir.dt.float16
GEN_NP = np.float16
MLO_SCALE = 1024.0            # 2^10, exact

ROWS = N // NCORES            # 1024 i's per core
NBLK = N // P                 # 64 j-blocks
CHUNK = 512                   # psum bank free-dim limit
NDIAG = ROWS // P             # 8 diagonal blocks per core

# input layout (f32 container cols)
OFF_EBC = 0                   # e bcast, fp16 pairs packed: 512 f32 cols
OFF_NEB = 512                 # -e_j bias cols, f32: 64
OFF_BSL = 576                 # bce slices o/l/w: 3*8 = 24
OFF_WST = 600                 # stationaries, fp16 packed: 12*64/2 = 384
OFF_MSK = 984                 # strict-lower-tri mask, fp16 packed: 64
W_IN = 1048

# per-position generation engine: 'A' = ACT single-instr, 'D' = DVE 2-instr,
# 'P' = DVE add + Pool AND. Spread evenly; last two positions DVE-ish so ACT
# is free for the psum copies at the tail.
N_ACT, N_POOL = 27, 0


def _gen_schedule():
    sched = {}
    pool_left, act_left = N_POOL, N_ACT
    for p in range(NBLK):
        if (p * N_POOL) // NBLK != ((p + 1) * N_POOL) // NBLK and pool_left:
            sched[p] = "P"
            pool_left -= 1
    rest = [p for p in range(NBLK) if p not in sched]
    for i, p in enumerate(rest):
        if (i * N_ACT) // len(rest) != ((i + 1) * N_ACT) // len(rest) and act_left:
            sched[p] = "A"
            act_left -= 1
        else:
            sched[p] = "D"
    # last two blocks on different engines so the drain overlaps
    sched[NBLK - 2] = "A"
    sched[NBLK - 1] = "D"
    return sched


GEN_SCHED = _gen_schedule()


def build_program():
    nc = bacc.Bacc(None)
    inp = nc.dram_tensor("inp", [P, W_IN], F32, kind="ExternalInput")
    gh = nc.dram_tensor("gh", [4, ROWS], F32, kind="ExternalOutput")
    bco = nc.dram_tensor("bco", [P, 1], F32, kind="ExternalOutput")

    with tile.TileContext(nc) as tc, ExitStack() as ctx:
        const = ctx.enter_context(tc.tile_pool(name="const", bufs=1))
        work = ctx.enter_context(tc.tile_pool(name="work", bufs=10))
        ps = ctx.enter_context(tc.tile_pool(name="ps", bufs=1, space="PSUM"))
        outp = ctx.enter_context(tc.tile_pool(name="outp", bufs=1))

        inpt = const.tile([P, W_IN], F32, tag="inpt")
        # two DMAs: first covers everything the generation loop reads early
        nc.sync.dma_start(out=inpt[:, :OFF_WST], in_=inp[:, :OFF_WST])
        nc.sync.dma_start(out=inpt[:, OFF_WST:], in_=inp[:, OFF_WST:])

        ebc = inpt[:, OFF_EBC : OFF_EBC + ROWS // 2].bitcast(GEN_DT)
        nebt = inpt[:, OFF_NEB : OFF_NEB + NBLK]
        wst = inpt[:, OFF_WST : OFF_WST + 384].bitcast(GEN_DT)   # [P, 768]
        maskt = inpt[:, OFF_MSK : OFF_MSK + 64].bitcast(GEN_DT)  # [P, 128]

        # BCE partial over this core's slice, in the DMA/warmup shadow:
        # softplus(x) = relu(x) + ln(1 + exp(-|x|))
        sl = ROWS // P
        ot = inpt[:, OFF_BSL : OFF_BSL + sl]
        lt = inpt[:, OFF_BSL + sl : OFF_BSL + 2 * sl]
        wt = inpt[:, OFF_BSL + 2 * sl : OFF_BSL + 3 * sl]
        sp = outp.tile([P, sl], F32, tag="sp")
        ol = outp.tile([P, sl], F32, tag="ol")
        tmp = outp.tile([P, sl], F32, tag="tmp")
        nc.scalar.activation(out=tmp, in_=ot, func=mybir.ActivationFunctionType.Abs)
        nc.scalar.activation(
            out=tmp, in_=tmp, func=mybir.ActivationFunctionType.Exp, scale=-1.0
        )
        nc.scalar.activation(
            out=tmp, in_=tmp, func=mybir.ActivationFunctionType.Ln, bias=1.0
        )
        # relu on DVE (max with 0) keeps ACT's queue shorter
        nc.vector.tensor_scalar(
            out=sp, in0=ot, scalar1=0.0, scalar2=None, op0=mybir.AluOpType.max
        )
        nc.vector.tensor_add(out=sp, in0=sp, in1=tmp)
        nc.vector.tensor_mul(out=ol, in0=ot, in1=lt)
        nc.vector.tensor_sub(out=sp, in0=sp, in1=ol)
        nc.vector.tensor_mul(out=sp, in0=sp, in1=wt)
        br = outp.tile([P, 1], F32, tag="br")
        nc.vector.reduce_sum(out=br, in_=sp, axis=mybir.AxisListType.X)
        nc.sync.dma_start(out=bco[:], in_=br)

        # one psum bank, 2 accumulators [4, 512] at partition offsets 0/32,
        # one per 512-chunk; distinct tile_position col groups overlap on PE
        # constant sign-clear mask for the Pool-engine ANDs (Pool has no
        # tensor_scalar on V3, only tensor_tensor)
        andm = const.tile([P, CHUNK], mybir.dt.int32, tag="andm")
        nc.gpsimd.memset(andm, 0x7FFF7FFF)

        bank = ps.tile([128, CHUNK], F32, tag="bank")
        started = {0: False, 32: False}

        def emit_mm(X, out_sl, stat, mov, last):
            # start=True exactly once per accumulator: it marks the whole 2KB
            # psum row pending-zero; later matmuls touching still-pending
            # bytes overwrite instead of accumulating.
            st_flag = not started[X]
            started[X] = True
            nc.tensor.matmul(
                bank[X : X + 4, out_sl],
                stat,
                mov,
                start=st_flag,
                stop=last,
                tile_position=(0, X),
                skip_group_check=True,
            )

        for p in range(NBLK):
            b = work.tile([P, ROWS], GEN_DT, tag="b")
            kind = GEN_SCHED[p]
            if kind == "A":
                nc.scalar.activation(
                    out=b,
                    in_=ebc,
                    func=mybir.ActivationFunctionType.Abs,
                    bias=nebt[:, p : p + 1],
                    scale=1.0,
                )
            else:
                nc.vector.tensor_scalar(
                    out=b,
                    in0=ebc,
                    scalar1=nebt[:, p : p + 1],
                    scalar2=None,
                    op0=mybir.AluOpType.add,
                )
                bi = b.bitcast(mybir.dt.int32)
                if kind == "P":
                    nc.gpsimd.tensor_tensor(
                        out=bi, in0=bi, in1=andm, op=mybir.AluOpType.bitwise_and
                    )
                else:
                    nc.vector.tensor_scalar(
                        out=bi,
                        in0=bi,
                        scalar1=0x7FFF7FFF,
                        scalar2=None,
                        op0=mybir.AluOpType.bitwise_and,
                    )
            stA = wst[:, 12 * p : 12 * p + 4]
            stB = wst[:, 12 * p + 4 : 12 * p + 8]
            stM = wst[:, 12 * p + 8 : 12 * p + 12]
            last = p == NBLK - 1
            if p % 8 != 0:
                # uniform sign block: one matmul per 512-chunk
                emit_mm(0, slice(0, CHUNK), stA, b[:, 0:CHUNK], last)
                emit_mm(32, slice(0, CHUNK), stA, b[:, CHUNK : 2 * CHUNK], last)
            else:
                # diagonal block t: strip of mixed signs at i-local 128t
                t = p // 8
                tq, toff = t // 4, 128 * (t % 4)
                mb = work.tile([P, P], GEN_DT, tag="mb")
                # Pool engine (idle otherwise) handles the small masked copy
                nc.gpsimd.tensor_tensor(
                    out=mb, in0=b[:, CHUNK * tq + toff : CHUNK * tq + toff + P],
                    in1=maskt, op=mybir.AluOpType.mult,
                )
                Xs = 0 if tq == 0 else 32
                Xo = 32 if tq == 0 else 0
                oc = slice(0, CHUNK) if tq == 1 else slice(CHUNK, 2 * CHUNK)
                # other chunk: all i right of strip -> +1 (B), left -> -1 (A)
                emit_mm(Xo, slice(0, CHUNK), stB if tq == 0 else stA, b[:, oc], last)
                if toff > 0:
                    emit_mm(Xs, slice(0, toff), stA,
                            b[:, CHUNK * tq : CHUNK * tq + toff], last)
                emit_mm(Xs, slice(toff, toff + P), stA,
                        b[:, CHUNK * tq + toff : CHUNK * tq + toff + P], last)
                emit_mm(Xs, slice(toff, toff + P), stM, mb, last)
                if toff + P < CHUNK:
                    emit_mm(Xs, slice(toff + P, CHUNK), stB,
                            b[:, CHUNK * tq + toff + P : CHUNK * (tq + 1)], last)

        # psum -> sbuf: ACT and DVE each copy one chunk in parallel into
        # SEPARATE tiles (shared tile would serialize on WAW tracking), each
        # followed by its own DMA so transfer latency overlaps the other copy
        uo0 = outp.tile([4, CHUNK], F32, tag="uo0")
        uo1 = outp.tile([4, CHUNK], F32, tag="uo1")
        nc.scalar.activation(
            out=uo0, in_=bank[0:4, :], func=mybir.ActivationFunctionType.Copy
        )
        nc.vector.tensor_copy(out=uo1, in_=bank[32:36, :])
        nc.sync.dma_start(out=gh[:, 0:CHUNK], in_=uo0)
        nc.sync.dma_start(out=gh[:, CHUNK : 2 * CHUNK], in_=uo1)

    nc.finalize()
    return nc


def make_in_maps(os_, ls_, ws_, eb, om, mhi, mlo):
    """Per-core packed inputs. All arrays already in GLOBAL SORTED order;
    eb/om/mhi/mlo are fp16 (mlo pre-scaled by MLO_SCALE)."""
    ebf = eb.astype(np.float32)
    omf = om.astype(np.float32)
    mhif = mhi.astype(np.float32)
    mlof = mlo.astype(np.float32)
    zero = np.zeros(P, np.float32)

    maskt = np.triu(np.ones((P, P), np.float32), 1).astype(GEN_NP)
    msk_pack = np.ascontiguousarray(maskt).view(np.float32)  # [128, 64]

    in_maps = []
    for c in range(NCORES):
        r = slice(c * ROWS, (c + 1) * ROWS)
        dlo, dhi = NDIAG * c, NDIAG * (c + 1)
        others = [k for k in range(NBLK) if not dlo <= k < dhi]
        perm, oi = [], 0
        for pp in range(NBLK):
            if pp % 8 == 0:
                perm.append(dlo + pp // 8)
            else:
                perm.append(others[oi])
                oi += 1

        neb = np.empty((NBLK, P), np.float32)
        wst = np.zeros((NBLK, 12, P), np.float32)
        for pp, jb in enumerate(perm):
            js = slice(jb * P, (jb + 1) * P)
            neb[pp] = -ebf[js]
            o_, h_, l_ = omf[js], mhif[js], mlof[js]
            if dlo <= jb < dhi:
                wst[pp, 0:4] = [-o_, -h_, -l_, o_]        # A = Sminus
                wst[pp, 4:8] = [o_, h_, l_, o_]           # B = Splus
                wst[pp, 8:12] = [2 * o_, 2 * h_, 2 * l_, zero]  # M2
            else:
                s = 1.0 if jb < dlo else -1.0
                wst[pp, 0:4] = [s * o_, s * h_, s * l_, o_]
        wst16 = np.ascontiguousarray(
            wst.transpose(2, 0, 1).reshape(P, NBLK * 12).astype(GEN_NP)
        ).view(np.float32)  # [128, 384]

        ebc = np.broadcast_to(
            np.ascontiguousarray(eb[r]).view(np.float32), (P, ROWS // 2)
        )
        sl = ROWS // P
        inp = np.concatenate(
            [
                ebc,
                neb.T,
                os_[r].reshape(sl, P).T,
                ls_[r].reshape(sl, P).T,
                ws_[r].reshape(sl, P).T,
                wst16,
                msk_pack,
            ],
            axis=1,
        )
        in_maps.append({"inp": np.ascontiguousarray(inp, dtype=np.float32)})
    return in_maps


def combine(results, ob, eb, om, bce_sum):
    """Host O(N) finish in f64 from device partials (sorted order)."""
    n = float(N)
    G = np.concatenate([results[c]["gh"][0] for c in range(NCORES)]).astype(np.float64)
    Hhi = np.concatenate([results[c]["gh"][1] for c in range(NCORES)]).astype(np.float64)
    Hlo = np.concatenate([results[c]["gh"][2] for c in range(NCORES)]).astype(np.float64)
    sv = np.concatenate([results[c]["gh"][3] for c in range(NCORES)]).astype(np.float64)

    ob64 = ob.astype(np.float64)
    eb64 = eb.astype(np.float64)
    om64 = om.astype(np.float64)
    st = ob64 * G - Hhi - Hlo / MLO_SCALE
    # exact a-side row sums via prefix sums (sorted order)
    Wc = np.cumsum(om64)
    Pfx = np.cumsum(om64 * ob64)
    su = ob64 * (2 * Wc - Wc[-1]) - 2 * Pfx + Pfx[-1]

    S = om64.sum()
    u = su / n
    v = sv / n
    T_ab = (om64 * st).sum()
    P_uv = (om64 * u * v).sum()
    P_uu = (om64 * u * u).sum()
    P_vv = (om64 * v * v).sum()
    mA = (om64 * u).sum() / n
    mB = (om64 * v).sum() / n
    T_aa = 2.0 * S * (om64 * ob64 * ob64).sum() - 2.0 * (om64 * ob64).sum() ** 2
    T_bb = 2.0 * S * (om64 * eb64 * eb64).sum() - 2.0 * (om64 * eb64).sum() ** 2
    c1 = 2.0 * S - 4.0 * n
    c2 = 4.0 * n * n - 4.0 * n * S + S * S
    num = (T_ab + c1 * P_uv + c2 * mA * mB) / n**2
    denA = (T_aa + c1 * P_uu + c2 * mA * mA) / n**2
    denB = (T_bb + c1 * P_vv + c2 * mB * mB) / n**2
    disco = num / np.sqrt(denA * denB)
    bce_mean = bce_sum / n
    tot = bce_mean + LAM * disco
    return (np.float32(bce_mean), np.float32(disco), np.float32(tot))


def run(outputs, labels, event, weights, **spmd_kwargs):
    o = np.asarray(outputs, dtype=np.float32)
    l = np.asarray(labels, dtype=np.float32)
    e = np.asarray(event, dtype=np.float32)
    w = np.asarray(weights, dtype=np.float32)
    assert o.shape == (N,)

    nw = (w * np.float32(N) / w.sum(dtype=np.float32)).astype(np.float32)
    order = np.argsort(o, kind="stable")
    os_, ls_, es_, ws_, nws = o[order], l[order], e[order], w[order], nw[order]
    ob = os_.astype(GEN_NP)
    eb = es_.astype(GEN_NP)
    om = nws.astype(GEN_NP)
    m64 = om.astype(np.float64) * ob.astype(np.float64)
    mhi = m64.astype(GEN_NP)
    mlo = ((m64 - mhi.astype(np.float64)) * MLO_SCALE).astype(GEN_NP)

    nc = build_program()
    in_maps = make_in_maps(os_, ls_, ws_, eb, om, mhi, mlo)
    bkr = run_bass_kernel_spmd(nc, in_maps, list(range(NCORES)), **spmd_kwargs)
    bce_sum = float(
        sum(bkr.results[c]["bco"].astype(np.float64).sum() for c in range(NCORES))
    )
    return combine(bkr.results, ob, eb, om, bce_sum), bkr


def kernel(outputs, labels, event, weights):
    out, _ = run(outputs, labels, event, weights)
    return out
